# revision 1
# baseline (speedup 1.0000x reference)
"""Trainium2 Bass kernel for nn_AssociativeLIF (8-core data-parallel over batch).

Self-contained: hardcodes T=8, B=128, D=8192, NC=64 from the problem spec.

Math (per timestep, matching reference.py):
    i_pre = bs*i + x_t
    u     = g*v + i_pre            with g = bm/(1-bm)   (u = new_v/(1-bm))
    z     = u - rt                 rt = 1e5*refrac  (refrac forces z << threshold)
    s     = (z >= th2)             th2 = th/(1-bm)  == (new_v >= th and refrac==0)
    cf    = sum_k s[b, k*64+c]     (cluster scatter-sum; d = k*64+c since
                                    cluster_ids = arange(D) % 64)
    ns    = (cf/128 @ W.T) * gain  (one 128x128 block-diag matmul on PE)
    i_new = i_pre + ns broadcast over k
    v_out = (1-bm)*(z - th2*s), overwritten with -0.1 where refrac was active
    rt'   = relu(rt - 1e5), overwritten with 2e5 where s fired

Layout per core (batch shard of 16): partition p = b01*64 + c, free f = b_lo*128 + k
with the shard batch index b = b01*8 + b_lo and neuron d = k*64 + c.

Toolchain constraint: every instruction may carry at most ONE sync-wait, so the
program is arranged so each op introduces at most one unobserved semaphore
(observer micro-copies absorb DMA/ACT ticks), and the DMA count stays within
the 8 HWDGE + 4 SWDGE semaphore lanes so no lane is ever reused.
"""

import numpy as np

import sys

for _p in ("/opt/trn_rl_repo", "/opt/pypackages"):
    if _p not in sys.path:
        sys.path.append(_p)

from concourse import bass, bacc, mybir
from concourse.tile import TileContext
from concourse.bass_utils import run_bass_kernel_spmd

T, B, D = 8, 128, 8192
NC = 64
K = D // NC          # 128 neurons per cluster
NCORES = 8
BL = B // NCORES     # 16 batch per core
P = 128              # partitions
F = BL * D // P      # 1024 free elements
RHO = 1.0e5
XCH = 4              # timesteps per x-load DMA

F32 = mybir.dt.float32
AF = mybir.ActivationFunctionType
OP = mybir.AluOpType

LAST_EXEC_NS = None
LAST_RESULT = None


def _patch_tail_drain():
    """Split the kernel-tail drain into one drain per proc: the walrus in this
    env rejects instructions carrying more than one sync-wait."""
    import concourse.tile as tile_mod
    from concourse.vector_clock import ScopedClock, VectorClock

    if getattr(tile_mod.TileContext, "_ant_split_drain", False):
        return

    def _drain_and_barrier(self, tick_clock, wait_clock):
        gc = tick_clock.global_clock
        n = 27
        for p in range(n):
            try:
                val = gc[p]
            except Exception:
                break
            if val:
                d = self.nc.sync.drain()
                wait_clock.add_sem_waits(
                    d.ins,
                    ScopedClock(
                        {None: VectorClock([val if q == p else 0 for q in range(n)])}
                    ),
                )
        self.nc.all_engine_barrier()
        assert self.sems is not None
        popped = self.nc._tile_sem_poison_stack.pop()
        assert popped is self._sem_poison
        self.nc.clear_and_free_semaphores(list(self.sems.allocated().values()))
        self.nc.all_engine_barrier()

    tile_mod.TileContext._drain_and_barrier = _drain_and_barrier
    tile_mod.TileContext._ant_split_drain = True


def _build(bs: float, g: float, om: float, th2: float, cneg_val: float) -> bass.Bass:
    nc = bacc.Bacc(None, target_bir_lowering=False, debug=False, num_swdge_queues=4)

    x_ext = nc.declare_dram_parameter("x", [T // XCH, P, XCH * F], F32, isOutput=False)
    wm_ext = nc.declare_dram_parameter("wmat", [P, 2 * P], F32, isOutput=False)
    wmb_ext = nc.declare_dram_parameter(
        "wmatb", [P, 2 * P], mybir.dt.bfloat16, isOutput=False
    )
    out_exts = [
        nc.declare_dram_parameter(f"out{t}", [P, 2, F], F32, isOutput=True)
        for t in range(T)
    ]

    with TileContext(nc) as tc:
        with (
            tc.tile_pool(name="const", bufs=1) as cpool,
            tc.tile_pool(name="state", bufs=2) as spool,
            tc.tile_pool(name="work", bufs=3) as wpool,
            tc.tile_pool(name="xin", bufs=2) as xpool,
            tc.tile_pool(name="outs", bufs=8) as opool,
            tc.tile_pool(name="ps", bufs=4, space="PSUM") as ppool,
        ):
            wm = cpool.tile([P, 2 * P], F32, name="wm")
            nc.sync.dma_start(out=wm, in_=wm_ext[:, :])
            wmb = cpool.tile([P, 2 * P], mybir.dt.bfloat16, name="wmb")
            nc.sync.dma_start(out=wmb, in_=wmb_ext[:, :])
            nrho = cpool.tile([P, 1], F32, name="nrho")
            nc.vector.memset(nrho, -RHO)
            c2e5 = cpool.tile([P, F], F32, name="c2e5")
            nc.vector.memset(c2e5, 2.0 * RHO)
            cneg = cpool.tile([P, F], F32, name="cneg")
            nc.vector.memset(cneg, cneg_val)

            v = spool.tile([P, F], F32, name="v0", tag="v")
            nc.vector.memset(v, 0.0)
            i = spool.tile([P, F], F32, name="i0", tag="iz")
            nc.vector.memset(i, 0.0)
            mpool_tiles = [
                spool.tile([P, F], F32, name=f"msk{j}", tag=f"msk{j}") for j in range(2)
            ]

            # dummy matmul so later matmuls don't need to wait on the wmat DMA
            dps = ppool.tile([P, 1], F32, name="dps", tag="dps", bufs=1)
            dps2 = ppool.tile([P, 1], F32, name="dps2", tag="dps2", bufs=1)
            nc.tensor.matmul(dps, wm[:, 0:P], wm[:, 0:1], start=True, stop=True)

            xbufs = []
            for ci in range(T // XCH):
                xb = xpool.tile([P, XCH * F], F32, name=f"xb{ci}", tag="xb", bufs=2)
                if ci == 0:
                    # split the first chunk so t=0 compute starts after 512KB
                    nc.sync.dma_start(out=xb[:, 0:F], in_=x_ext[0][:, 0:F])
                    nc.sync.dma_start(
                        out=xb[:, F : XCH * F], in_=x_ext[0][:, F : XCH * F]
                    )
                else:
                    nc.sync.dma_start(out=xb, in_=x_ext[ci])
                xbufs.append(xb)

            s_hist = [None, None]          # s_{t-1}, s_{t-2}
            msk = None                     # refrac mask: s_{t-1} | s_{t-2}
            for t in range(T):
                xt = xbufs[t // XCH][:, (t % XCH) * F : (t % XCH + 1) * F]

                last = t == T - 1
                sv = opool.tile([P, 2 * F], F32, name=f"sv{t}", tag="sv", bufs=8)
                s = sv[:, 0:F]
                vo = sv[:, F : 2 * F]
                if t > 0:
                    bv = wpool.tile([P, F], F32, name=f"bv{t}", tag="bv")
                    i_pre = wpool.tile([P, F], F32, name=f"ip{t}", tag="ip")
                    u = wpool.tile([P, F], F32, name=f"u{t}", tag="u")
                else:
                    i_pre = xt             # i=0, v=0 at t=0: i_pre = x, u = x
                    u = xt
                if not last:
                    cf = wpool.tile([P, NC // 8], F32, name=f"cf{t}", tag="cf", bufs=8)
                    cfb = wpool.tile(
                        [P, NC // 8], mybir.dt.bfloat16, name=f"cfb{t}", tag="cfb", bufs=8
                    )
                    i2 = ppool.tile([P, F], F32, name=f"i{t + 1}", tag="i2", bufs=3)
                    i2v = i2.rearrange("p (bl k) -> p bl k", k=K)
                s2 = wpool.tile([P, F], F32, name=f"s2{t}", tag="s2")
                e = wpool.tile([P, F], F32, name=f"e{t}", tag="e")

                HF = F // 2
                for h in range(2):
                    fh = slice(h * HF, (h + 1) * HF)
                    hb = slice(h * 4, (h + 1) * 4)
                    if t > 0:
                        # membrane potential (pre-scaled by 1/(1-bm))
                        nc.scalar.activation(bv[:, fh], v[:, fh], AF.Copy, scale=g)
                        nc.vector.tensor_tensor(
                            i_pre[:, fh], i[:, fh], xt[:, fh], op=OP.add
                        )
                    if not last:
                        # i2 += bs * i_pre (PE, fp32 diag)
                        nc.tensor.matmul(
                            i2[:, fh], wm[:, P : 2 * P], i_pre[:, fh],
                            start=True, stop=False,
                        )
                    if t > 0:
                        nc.vector.tensor_tensor(
                            u[:, fh], bv[:, fh], i_pre[:, fh], op=OP.add
                        )
                        nc.vector.copy_predicated(
                            u[:, fh], msk[:, fh].bitcast(mybir.dt.uint32),
                            cneg[:, fh],
                        )                                            # refrac lanes
                    nc.vector.tensor_scalar(
                        s[:, fh], u[:, fh], th2, None, op0=OP.is_ge
                    )
                    if not last:
                        s3 = s[:, fh].rearrange("p (bl k) -> p bl k", k=K)
                        nc.vector.tensor_reduce(
                            cf[:, hb], s3, axis=mybir.AxisListType.X, op=OP.add
                        )
                        nc.vector.tensor_copy(cfb[:, hb], cf[:, hb])
                        # i2 += bs * (cluster mix) broadcast over k (PE);
                        # bf16 hi+lo split of the mix matrix — exact since cf
                        # are small integers and hi*cf/lo*cf fit fp32
                        rhs_b = cfb[:, hb].unsqueeze(2).broadcast_to([P, 4, K])
                        nc.tensor.matmul(
                            i2v[:, hb], wmb[:, 0:P], rhs_b,
                            start=False, stop=False,
                        )
                        nc.tensor.matmul(
                            i2v[:, hb], wmb[:, P : 2 * P], rhs_b,
                            start=False, stop=True,
                        )
                    # v path
                    nc.scalar.activation(s2[:, fh], s[:, fh], AF.Copy, scale=th2)

                for h in range(2):
                    fh = slice(h * HF, (h + 1) * HF)
                    nc.vector.tensor_tensor(
                        e[:, fh], u[:, fh], s2[:, fh], op=OP.subtract
                    )
                    nc.scalar.activation(vo[:, fh], e[:, fh], AF.Copy, scale=om)
                # refrac mask for t+1: s_t | s_{t-1}  (REF_T == 2)
                if not last and t > 0:
                    mt = mpool_tiles[t % 2]
                    nc.vector.tensor_tensor(mt, s, s_hist[0], op=OP.add)

                # one DMA for [s | v_out]; split across HWDGE/SWDGE lanes
                dst = out_exts[t][:, :, :]                           # [p, io, f]
                src_ap = sv.rearrange("p (io f) -> p io f", f=F)
                nc.sync.dma_start(out=dst, in_=src_ap)

                if not last:
                    msk = s if t == 0 else mpool_tiles[t % 2]
                    s_hist = [s, s_hist[0]]
                    v, i = vo, i2

    nc.finalize()
    return nc


def _ensure_ntff_hook():
    """Register the NTFF profiling hook if the image's antenv lacks it."""
    import types

    try:
        from antenv.axon_hooks import get_axon_ntff_profile_hook  # noqa: F401

        return
    except ImportError:
        pass
    try:
        import antenv
        from trn_agent_boot.trn_boot import _ntff_profile_via_ctypes

        mod = types.ModuleType("antenv.axon_hooks")
        _h = [None]
        mod.set_axon_ntff_profile_hook = lambda h: _h.__setitem__(0, h)
        mod.get_axon_ntff_profile_hook = lambda: _h[0]
        sys.modules["antenv.axon_hooks"] = mod
        antenv.axon_hooks = mod
        mod.set_axon_ntff_profile_hook(
            _ntff_profile_via_ctypes("/opt/axon/libaxon_pjrt.so")
        )
    except Exception as e:  # profiling is best-effort
        print(f"ntff hook registration failed: {e}", file=sys.stderr)


def _sigmoid64(x):
    return (1.0 / (1.0 + np.exp(-np.asarray(x, np.float64)))).astype(np.float32)


def kernel(
    current_in,
    threshold_raw,
    beta_mem_raw,
    beta_syn_raw,
    neighbor_weights,
    cluster_gain,
    cluster_ids,
):
    x = np.asarray(current_in, np.float32)
    assert x.shape == (T, B, D)

    bm = np.float32(np.clip(_sigmoid64(beta_mem_raw), 0.8, 0.98))
    bs = np.float32(_sigmoid64(beta_syn_raw))
    th_vec = np.clip(np.asarray(threshold_raw, np.float32), 0.05, 0.5)
    th = np.float32(th_vec.flat[0])
    om = np.float32(1.0) - bm                 # 1-bm in f32, as reference
    g = np.float32(bm / om)
    th2 = np.float32(th / om)
    W = _sigmoid64(neighbor_weights)          # [64,64] f32
    gain = np.asarray(cluster_gain, np.float32)

    # mixing matrix including the /K normalization: ns[b,c] = sum_c' cf_raw[b,c'] * Mm[c',c]
    Mm = (W.T * gain[None, :]).astype(np.float32) / np.float32(K)
    MmS = (Mm * bs).astype(np.float32)
    wmat = np.zeros((P, 2 * P), np.float32)
    wmat[:NC, :NC] = MmS
    wmat[NC : 2 * NC, NC : 2 * NC] = MmS
    wmat[:, P : 2 * P] = np.diag(np.full(P, bs, np.float32))

    bd = np.zeros((P, P), np.float32)
    bd[:NC, :NC] = MmS
    bd[NC : 2 * NC, NC : 2 * NC] = MmS
    import ml_dtypes

    hi = bd.astype(ml_dtypes.bfloat16)
    lo = (bd - hi.astype(np.float32)).astype(ml_dtypes.bfloat16)
    wmatb = np.concatenate([hi, lo], axis=1)

    cneg_val = float(np.float32(np.float32(-0.1) / om))
    nc = _build(float(bs), float(g), float(om), float(th2), cneg_val)

    in_maps = []
    for ci in range(NCORES):
        xc = x[:, ci * BL : (ci + 1) * BL, :]            # [T,16,8192]
        xt = xc.reshape(T, 2, 8, K, NC)                  # [t,b01,b_lo,k,c]
        xt = xt.transpose(0, 1, 4, 2, 3).reshape(T, P, F)  # [t,p,f]
        xt = np.ascontiguousarray(
            xt.reshape(T // XCH, XCH, P, F).transpose(0, 2, 1, 3)
        ).reshape(T // XCH, P, XCH * F)
        in_maps.append({"x": xt, "wmat": wmat, "wmatb": wmatb})

    import os

    trace = os.environ.get("BASS_KERNEL_TRACE", "0") == "1"
    if trace:
        _ensure_ntff_hook()
    res = run_bass_kernel_spmd(
        nc, in_maps, core_ids=list(range(NCORES)), trace=trace
    )
    global LAST_EXEC_NS, LAST_RESULT
    LAST_EXEC_NS = res.exec_time_ns
    LAST_RESULT = res

    ss = np.empty((T, B, D), np.float32)
    vt = np.empty((T, B, D), np.float32)
    for ci in range(NCORES):
        rm = res.results[ci]
        o = np.stack([np.asarray(rm[f"out{t}"]) for t in range(T)])  # [T,128,2,1024]
        o = o.transpose(2, 0, 1, 3).reshape(2, T, 2, NC, 8, K)
        o = o.transpose(0, 1, 2, 4, 5, 3)                # [io,t,b01,b_lo,k,c]
        o = o.reshape(2, T, BL, D)
        ss[:, ci * BL : (ci + 1) * BL, :] = o[0]
        vt[:, ci * BL : (ci + 1) * BL, :] = o[1]
    return ss, vt


if __name__ == "__main__":
    rng = np.random.default_rng(0)
    out = kernel(
        current_in=rng.standard_normal((T, B, D), dtype=np.float32),
        threshold_raw=np.full((D,), 0.12, np.float32),
        beta_mem_raw=np.float32(np.log(0.85 / (1 - 0.85 + 1e-6))),
        beta_syn_raw=np.float32(0.0),
        neighbor_weights=np.zeros((NC, NC), np.float32),
        cluster_gain=np.full((NC,), 0.8, np.float32),
        cluster_ids=(np.arange(D) % NC).astype(np.int32),
    )
    print(out[0].shape, out[1].shape)

